# revision 58
# baseline (speedup 1.0000x reference)
"""Grouped-query attention (B=1, S=2048, HID=4096, 32 q-heads / 8 kv-heads,
D=128, RoPE, additive causal mask) on 8 Trainium2 NeuronCores.

Sharding: tensor-parallel over heads. Core c owns 4 q-heads (columns
512c:512c+512 of Wq), kv-head c (columns 128c:128c+128 of Wk/Wv), and rows
512c:512c+512 of Wo. Each core emits a full-shape partial of the output
projection in bf16; the host sums the 8 partials (the "all-reduce" of the
row-sharded Wo matmul).

Fully software-pipelined per-core schedule over 512-query slices j = 0..3,
window order A0 A1 | B0B1+C0 | A2 | B2+C1 | A3 | B3+C2+C3:

  A(j)  x^T chunks streamed from DRAM interleaved with the (bf16) combined
        QKV weight; weights cast up to f32r on the Activation engine; the
        first chunk runs straight off the bf16 weights so the PE starts
        ~2.5us in. Q^T/K^T/V^T accumulate in 6 PSUM banks; at the drain, the
        pool close is gated by the last PSUM read, so q0/K/q1 RoPE
        multiplies run PSUM-first on DVE while q2/q3 are staged out through
        Act copies (straight + half-swapped) and RoPE'd on the idle GpSimd
        engine; V transposes on the PE fill the drain latency.
  B(j)  attention (scores transposed so exp lands in P^T layout; P tiles
        bf16; pair-summed softmax denominator: DVE adds + ones-matmuls on
        half the tiles; 1/l broadcast across partitions on GpSimd; diagonal
        tiles t>=4j+2 at half width; depth-4 software pipeline streamed
        continuously across head/slice boundaries). Interleaved as PE
        filler: the previous slice's output projection C(j-1) -- its bf16
        matmuls keep the PE busy while the Activation engine works through
        exp tiles -- plus x prefetches for A(j+1). B0 and B1 share one
        window so B1+C0 work hides B0's Act-bound portion.

PSUM budget per window: A: 6 accumulators (+2 transpose banks at drains);
B: 3 score + 1 lsum + 2 out + 2 yproj banks.

Measured (TimelineSim cost model, matches the HW-exec contract of test.py):
606us baseline -> 400us; hardware rel err vs the f32 reference: 3.3e-3.
"""
import os

import numpy as np
import ml_dtypes
from contextlib import ExitStack

import concourse.tile as tile
from concourse import bacc, mybir
from concourse.bass_utils import run_bass_kernel_spmd
from concourse.masks import make_identity

F32 = mybir.dt.float32
F32R = mybir.dt.float32r
BF16 = mybir.dt.bfloat16
EXP = mybir.ActivationFunctionType.Exp

S = 2048
HID = 4096
D = 128
NCORES = 8
NHQ = 4                      # q heads per core
SCALE = float(D) ** -0.5
ST = S // 128                # 16 s-tiles
SL = S // 512                # 4 s-slices
KT = HID // 128              # 32 hidden k-tiles
NO = HID // 512              # 8 output column slices
NG = KT // 4                 # 8 hidden chunk-groups of 4 k-tiles
CHK = 4                      # k-tiles per fetch chunk
NCH = KT // CHK              # 16 fetch chunks per slice
DEPTH = 4                    # phase-B software pipeline depth

_NC_CACHE = {}


def round_fp32r(a: np.ndarray) -> np.ndarray:
    """Round fp32 to fp32r (1s+8e+11m, top 20 bits), round-to-nearest-even."""
    u = np.ascontiguousarray(a, dtype=np.float32).view(np.uint32)
    low = u & np.uint32(0xFFF)
    base = u & np.uint32(0xFFFFF000)
    lsb = (u >> np.uint32(12)) & np.uint32(1)
    up = (low > np.uint32(0x800)) | ((low == np.uint32(0x800)) & (lsb == np.uint32(1)))
    return (base + np.where(up, np.uint32(0x1000), np.uint32(0))).view(np.float32)


def build_nc():
    nc = bacc.Bacc("TRN2", target_bir_lowering=False, debug=False,
                   num_devices=NCORES)
    xT = nc.dram_tensor("xT", [HID, S], F32R, kind="ExternalInput").ap()
    wb = nc.dram_tensor("wb", [HID, 128 * NHQ + 2 * D], BF16,
                        kind="ExternalInput").ap()
    wob = nc.dram_tensor("wob", [128 * NHQ, HID], BF16, kind="ExternalInput").ap()
    mask4 = nc.dram_tensor("mask4", [4, 128, 512], BF16, kind="ExternalInput").ap()
    cosT = nc.dram_tensor("cosT", [128, S], BF16, kind="ExternalInput").ap()
    sinTf = nc.dram_tensor("sinTf", [128, S], BF16, kind="ExternalInput").ap()
    ones = nc.dram_tensor("ones", [128, 1], BF16, kind="ExternalInput").ap()
    xh = nc.dram_tensor("xh", [512, 512], BF16, kind="ExternalInput").ap()
    y = nc.dram_tensor("y", [S, HID], BF16, kind="ExternalOutput").ap()

    with tile.TileContext(nc) as tc, ExitStack() as ctx:
        const = ctx.enter_context(tc.tile_pool(name="const", bufs=1))
        ones_sb = const.tile([128, 1], BF16)
        ident = const.tile([128, 128], F32)
        mask_sb = const.tile([128, 4, 512], BF16)
        kt_sl = [const.tile([128, 512], F32R, tag=f"kt{j}", name=f"kt{j}")
                 for j in range(SL)]
        v_sl = [const.tile([128, 512], BF16, tag=f"v{j}", name=f"v{j}")
                for j in range(SL)]
        wb_sb = const.tile([128, KT, 128 * NHQ + 2 * D], BF16)

        xtp = ctx.enter_context(tc.tile_pool(name="xtp", bufs=3))
        wkvc = ctx.enter_context(tc.tile_pool(name="wkvc", bufs=3))
        ropes = ctx.enter_context(tc.tile_pool(name="ropes", bufs=5))
        vtmp = ctx.enter_context(tc.tile_pool(name="vtmp", bufs=1))
        qtp = ctx.enter_context(tc.tile_pool(name="qtp", bufs=2))
        outTp = ctx.enter_context(tc.tile_pool(name="outTp", bufs=2))
        ptbp = ctx.enter_context(tc.tile_pool(name="ptbp", bufs=9))
        prp = ctx.enter_context(tc.tile_pool(name="prp", bufs=3))
        rbcp = ctx.enter_context(tc.tile_pool(name="rbcp", bufs=1))
        wop = ctx.enter_context(tc.tile_pool(name="wop", bufs=2))
        csp = ctx.enter_context(tc.tile_pool(name="csp", bufs=2))
        ysp = ctx.enter_context(tc.tile_pool(name="ysp", bufs=2))

        make_identity(nc, ident[:])

        wbr = wb.rearrange("(t p) f -> p t f", p=128)
        wor = wob.rearrange("(k p) o -> p k o", p=128)
        casts = {}
        cs_sl = {}
        sn_sl = {}
        qt_sl = {}
        outT_sl = {}
        wo_t = {}

        def _fetch(j, g):
            """DMA (j0: weights +) x chunk; cast weight chunk up to f32r."""
            sk = slice(CHK * g, CHK * (g + 1))
            if j == 0:
                nc.sync.dma_start(out=wb_sb[:, sk, :], in_=wbr[:, sk, :])
            xt = xtp.tile([128, CHK, 512], F32R, tag="xt", name="xt")
            nc.sync.dma_start(
                out=xt[:],
                in_=xT[128 * CHK * g:128 * CHK * (g + 1),
                       512 * j:512 * (j + 1)]
                .rearrange("(t p) m -> p t m", p=128),
            )
            if j == 0 and 3 <= g <= 4:
                # constants are first needed at the j0 RoPE drain / B(0);
                # keep them off the prologue's critical DMA path
                if g == 3:
                    nc.sync.dma_start(out=ones_sb[:], in_=ones[:])
                else:
                    nc.sync.dma_start(out=mask_sb[:],
                                      in_=mask4.rearrange("t p q -> p t q"))
            if g == 5:
                cs_sl[j] = csp.tile([128, 512], BF16, tag="cs", name="cs")
                nc.sync.dma_start(out=cs_sl[j][:],
                                  in_=cosT[:, 512 * j:512 * (j + 1)])
            elif g == 6:
                sn_sl[j] = csp.tile([128, 512], BF16, tag="sn", name="sn")
                nc.sync.dma_start(out=sn_sl[j][:],
                                  in_=sinTf[:, 512 * j:512 * (j + 1)])
            # weight cast on Act: DVE is busy with RoPE drains and must
            # not delay the next slice's first matmuls
            wc = wkvc.tile([128, CHK, 128 * NHQ + 2 * D], F32R, tag="wc",
                           name="wc")
            nc.scalar.copy(wc[:], wb_sb[:, sk, :])
            casts[(j, g)] = [(xt, wc[:, :, 0:512], wc[:, :, 512:640],
                              wc[:, :, 640:768], 0, CHK)]

        def _prefetch_wo(jm1, n):
            wt = wop.tile([128, NHQ, 512], BF16, tag="wo", name="wo")
            nc.sync.dma_start(out=wt[:], in_=wor[:, :, 512 * n:512 * (n + 1)])
            wo_t[(jm1, n)] = wt

        def _emit_cblock(jm1, n, psC, split_dma=False, act_copies=False):
            """Output projection for q-slice jm1, output columns 512n:512n+512."""
            wt = wo_t.pop((jm1, n))
            ys = ysp.tile([128, 4, 512], BF16, tag="ys", name="ys")
            for sq2 in range(4):
                yp = psC.tile([128, 512], F32, tag="yp", name="yp")
                for h in range(NHQ):
                    nc.tensor.matmul(
                        yp[:], outT_sl[(h, jm1)][:, 128 * sq2:128 * (sq2 + 1)],
                        wt[:, h, :], start=(h == 0), stop=(h == NHQ - 1))
                if sq2 % 2 == 0 and not act_copies:
                    nc.vector.tensor_copy(ys[:, sq2, :], yp[:])
                else:
                    nc.scalar.copy(ys[:, sq2, :], yp[:])
                if split_dma:
                    sq = 4 * jm1 + sq2
                    nc.sync.dma_start(
                        out=y[128 * sq:128 * (sq + 1),
                              512 * n:512 * (n + 1)],
                        in_=ys[:, sq2, :])
            if not split_dma:
                nc.sync.dma_start(
                    out=y[512 * jm1:512 * (jm1 + 1), 512 * n:512 * (n + 1)]
                    .rearrange("(t p) o -> p t o", p=128),
                    in_=ys[:])

        # ---------------- phase B emission helpers ---------------------------
        bstate = {}

        def _off(j, t):
            # diagonal tiles t >= 4j+2 only need the upper half q range
            return 256 if t >= 4 * j + 2 else 0

        def _bseq(j, h):
            diag = [4 * j, 4 * j + 1, 4 * j + 2, 4 * j + 3]
            nond = list(range(4 * j))
            # h=0 starts with non-diagonal tiles: they only touch earlier
            # slices' K/V, so B(j) can begin while slice j still drains
            return nond + diag if h == 0 else diag + nond

        def _blead(j, h, pos, psB, psL, psO):
            seq = _bseq(j, h)
            t = seq[pos]
            if pos == 0:
                bstate[(j, h)] = {
                    "lps": psL.tile([1, 512], F32, tag="lps", name="lps"),
                    "ops": psO.tile([128, 512], F32, tag="ops", name="ops"),
                    "pts": {}, "pairs": {},
                }
            st = bstate[(j, h)]
            o = _off(j, t)
            scp = psB.tile([128, 512], F32, tag="scp", name="scp")
            nc.tensor.matmul(
                scp[:, o:512], kt_sl[t // 4][:, 128 * (t % 4):128 * (t % 4 + 1)],
                qt_sl[(h, j)][:, o:512],
                start=True, stop=True, skip_group_check=True)
            if t >= 4 * j:  # diagonal block: additive mask
                nc.vector.tensor_add(
                    scp[:, o:512], scp[:, o:512], mask_sb[:, t - 4 * j, o:512])
            ptb = ptbp.tile([128, 512], BF16, tag="ptb", name="ptb")
            nc.scalar.activation(ptb[:, o:512], scp[:, o:512],
                                 EXP, bias=0.0, scale=SCALE)
            st["pts"][t] = ptb
            if pos % 2 == 1:
                t0 = seq[pos - 1]
                o2 = _off(j, t0)  # pair partners share width by construction
                pp = prp.tile([128, 512], BF16, tag="pp", name="pp")
                nc.vector.tensor_add(pp[:, o2:512], st["pts"][t0][:, o2:512],
                                     st["pts"][t][:, o2:512])
                st["pairs"][pos // 2] = pp

        def _btrail(j, h, pos):
            seq = _bseq(j, h)
            t = seq[pos]
            st = bstate[(j, h)]
            o = _off(j, t)
            nc.tensor.matmul(
                st["ops"][:, o:512], v_sl[t // 4][:, 128 * (t % 4):128 * (t % 4 + 1)],
                st["pts"].pop(t)[:, o:512],
                start=(pos == 0), stop=(pos == len(seq) - 1),
                skip_group_check=True)
            if pos % 2 == 1:
                o2 = _off(j, seq[pos - 1])
                nc.tensor.matmul(
                    st["lps"][:, o2:512], ones_sb[:],
                    st["pairs"].pop(pos // 2)[:, o2:512],
                    start=(pos == 1), stop=(pos == len(seq) - 1),
                    skip_group_check=True)
            if pos == len(seq) - 1:
                rsb = rbcp.tile([1, 512], F32, tag="rsb", name="rsb")
                nc.vector.reciprocal(rsb[:], st["lps"][:])
                rb = rbcp.tile([128, 512], F32, tag="rb", name="rb")
                nc.gpsimd.partition_broadcast(rb[:], rsb[:])
                ot = outTp.tile([128, 512], BF16, tag=f"outT{h}",
                                name=f"outT{h}")
                nc.vector.tensor_mul(ot[:], st["ops"][:], rb[:])
                outT_sl[(h, j)] = ot
                del bstate[(j, h)]

        def _emit_awindow(jlist, wname):
            """Projection chunks + drains for the slices in jlist."""
            stA = ExitStack()
            psA = stA.enter_context(
                tc.tile_pool(name=f"psA{wname}", bufs=1, space="PSUM"))
            chunks = [(j, g) for j in jlist for g in range(NCH)]
            for ci, (j, g) in enumerate(chunks):
                if g == 0:
                    qps = [psA.tile([128, 512], F32, tag=f"qps{f}",
                                    name=f"qps{f}") for f in range(NHQ)]
                    kps = psA.tile([128, 512], F32, tag="kps", name="kps")
                    vps = psA.tile([128, 512], F32, tag="vps", name="vps")
                if ci + 2 < len(chunks) and chunks[ci + 2] not in casts:
                    _fetch(*chunks[ci + 2])
                for xt, wqc, wkc, wvc, k0, nk in casts.pop((j, g)):
                    for kk in range(nk):
                        k = CHK * g + k0 + kk
                        rhs = xt[:, kk, :]
                        st, sp = (k == 0), (k == KT - 1)
                        if not sp:
                            for f in range(NHQ):
                                nc.tensor.matmul(
                                    qps[f][:],
                                    wqc[:, kk, 128 * f:128 * (f + 1)],
                                    rhs, start=st, stop=sp)
                            nc.tensor.matmul(kps[:], wkc[:, kk, :], rhs,
                                             start=st, stop=sp)
                            nc.tensor.matmul(vps[:], wvc[:, kk, :], rhs,
                                             start=st, stop=sp)
                        else:
                            # last k-tile: K/V first so their drains (kt
                            # RoPE on DVE, vts copy on Act) start earlier
                            nc.tensor.matmul(kps[:], wkc[:, kk, :], rhs,
                                             start=st, stop=sp)
                            nc.tensor.matmul(vps[:], wvc[:, kk, :], rhs,
                                             start=st, stop=sp)
                            for f in range(NHQ):
                                nc.tensor.matmul(
                                    qps[f][:],
                                    wqc[:, kk, 128 * f:128 * (f + 1)],
                                    rhs, start=st, stop=sp)
                if g == NCH - 1:
                    # Drains. The pool close (and thus the next window) waits
                    # on the last PSUM read, so emit all PSUM-reading muls
                    # first (q3's on the idle GpSimd engine, in parallel) and
                    # defer the rotate-half adds; V transposes on the PE fill
                    # the drain latency.
                    cs = cs_sl[j][:]
                    sn = sn_sl[j][:]
                    vts = vtmp.tile([128, 512], F32, tag="vts", name="vts")
                    nc.scalar.copy(vts[:], vps[:])
                    for h in range(NHQ):
                        qt_sl[(h, j)] = qtp.tile([128, 512], F32R,
                                                 tag=f"qt{h}", name=f"qt{h}")

                    def _muls(eng, dst, src):
                        rot = ropes.tile([128, 512], BF16, tag="rot",
                                         name="rot")
                        eng.tensor_mul(dst[:], src[:], cs)
                        eng.tensor_mul(rot[0:64, :], src[64:128, :],
                                       sn[0:64, :])
                        eng.tensor_mul(rot[64:128, :], src[0:64, :],
                                       sn[64:128, :])
                        return rot

                    # GpSimd cannot read PSUM, and SBUF-SBUF ops need equal
                    # base partitions: stage q2/q3 twice via Act copies from
                    # PSUM (straight + half-swapped), then RoPE them on the
                    # idle Pool engine with fully-aligned ops. This takes
                    # their PSUM reads off the DVE critical path that gates
                    # the pool close (and thus the next window).
                    def _stage_pool_rope(f):
                        qu = vtmp.tile([128, 512], BF16, tag=f"q{f}u",
                                       name=f"q{f}u")
                        qs = vtmp.tile([128, 512], BF16, tag=f"q{f}s",
                                       name=f"q{f}s")
                        nc.scalar.copy(qu[:], qps[f][:])
                        nc.scalar.copy(qs[0:64, :], qps[f][64:128, :])
                        nc.scalar.copy(qs[64:128, :], qps[f][0:64, :])
                        qtf = qt_sl[(f, j)]
                        rotf = ropes.tile([128, 512], BF16, tag="rot",
                                          name="rot")
                        nc.gpsimd.tensor_mul(qtf[:], qu[:], cs)
                        nc.gpsimd.tensor_mul(rotf[:], qs[:], sn)
                        nc.gpsimd.tensor_add(qtf[:], qtf[:], rotf[:])

                    rot0 = _muls(nc.vector, qt_sl[(0, j)], qps[0])
                    rotk = _muls(nc.vector, kt_sl[j], kps)
                    with tc.tile_pool(name=f"psAv{j}", bufs=2,
                                      space="PSUM") as psAv:
                        for t2 in range(4):
                            vtp = psAv.tile([128, 128], F32, tag="vtp",
                                            name="vtp")
                            nc.tensor.transpose(
                                vtp[:], vts[:, 128 * t2:128 * (t2 + 1)],
                                ident[:])
                            nc.scalar.copy(
                                v_sl[j][:, 128 * t2:128 * (t2 + 1)], vtp[:])
                    rot1 = _muls(nc.vector, qt_sl[(1, j)], qps[1])
                    _stage_pool_rope(3)
                    _stage_pool_rope(2)
                    nc.vector.tensor_add(qt_sl[(0, j)][:], qt_sl[(0, j)][:],
                                         rot0[:])
                    nc.vector.tensor_add(kt_sl[j][:], kt_sl[j][:], rotk[:])
                    nc.vector.tensor_add(qt_sl[(1, j)][:], qt_sl[(1, j)][:],
                                         rot1[:])
            stA.close()

        def _emit_bwindow(jlist, cfill_j, fetch_j, wname, trail_j=None):
            """Attention for slices in jlist, with output-projection blocks
            of slice cfill_j and x prefetches for slice fetch_j as PE
            filler."""
            stB = ExitStack()
            psB = stB.enter_context(
                tc.tile_pool(name=f"psB{wname}", bufs=3, space="PSUM"))
            psL = stB.enter_context(
                tc.tile_pool(name=f"psL{wname}", bufs=1, space="PSUM"))
            psO = stB.enter_context(
                tc.tile_pool(name=f"psO{wname}", bufs=2, space="PSUM"))
            psC = stB.enter_context(
                tc.tile_pool(name=f"psC{wname}", bufs=2, space="PSUM"))

            items = [(j, h, pos) for j in jlist for h in range(NHQ)
                     for pos in range(4 * j + 4)]
            fills = []
            if cfill_j is not None:
                for n in range(NO):
                    fills.append(("cblock", cfill_j, n))
                _prefetch_wo(cfill_j, 0)
            if fetch_j is not None:
                fills.append(("fetch", fetch_j, 0))
                fills.append(("fetch", fetch_j, 1))
            # C blocks of slice jlist[0]-1 may only start once that slice's
            # outT is complete; when filling with jlist[0]'s own C work
            # (merged B0+B1 window), wait for the first slice to trail out
            fill_start = 0
            if cfill_j is not None and cfill_j in jlist:
                fill_start = 4 * (4 * cfill_j + 4) + DEPTH
            every = max(1, (len(items) - fill_start) // max(1, len(fills) + 1))

            held = [f for f in fills if f[0] == "cblock"][-2:]
            for f in held:
                fills.remove(f)
            fi = 0
            for idx in range(len(items) + DEPTH):
                if idx < len(items):
                    _blead(*items[idx], psB, psL, psO)
                if (idx >= fill_start and fi < len(fills)
                        and (idx - fill_start) % every == every - 1):
                    kind, a1, a2 = fills[fi]
                    fi += 1
                    if kind == "cblock":
                        if a2 + 1 < NO:
                            _prefetch_wo(a1, a2 + 1)
                        _emit_cblock(a1, a2, psC)
                    else:
                        _fetch(a1, a2)
                if idx >= DEPTH:
                    _btrail(*items[idx - DEPTH])
            fills.extend(held)
            while fi < len(fills):
                kind, a1, a2 = fills[fi]
                fi += 1
                if kind == "cblock":
                    if a2 + 1 < NO:
                        _prefetch_wo(a1, a2 + 1)
                    # trailing blocks: keep their PSUM drains off the busy
                    # DVE so the next A window isn't held up
                    _emit_cblock(a1, a2, psC, act_copies=True)
                else:
                    _fetch(a1, a2)
            if trail_j is not None:
                # final slice's own output projection, inside this window
                _prefetch_wo(trail_j, 0)
                for n in range(NO):
                    if n + 1 < NO:
                        _prefetch_wo(trail_j, n + 1)
                    _emit_cblock(trail_j, n, psC, split_dma=(n >= NO - 2))
            stB.close()

        # ===================== main schedule =================================
        # A0 A1 | B0+B1 (+C0 fills) | A2 | B2 (+C1) | A3 | B3 (+C2) | C3
        # chunk (0,0) runs fully in bf16 straight off the resident bf16
        # weights and a small bf16 x slice -- no casts on the critical path,
        # so the PE starts ~2.5us in instead of ~7
        halfp = ctx.enter_context(tc.tile_pool(name="halfp", bufs=1))
        segs0 = []
        for hi in range(2):
            sk = slice(2 * hi, 2 * hi + 2)
            nc.sync.dma_start(out=wb_sb[:, sk, :], in_=wbr[:, sk, :])
            xth = halfp.tile([128, 2, 512], BF16, tag=f"xth{hi}",
                             name=f"xth{hi}")
            nc.sync.dma_start(
                out=xth[:],
                in_=xh[256 * hi:256 * (hi + 1), :]
                .rearrange("(t p) m -> p t m", p=128))
            segs0.append((xth, wb_sb[:, sk, 0:512], wb_sb[:, sk, 512:640],
                          wb_sb[:, sk, 640:768], 2 * hi, 2))
        casts[(0, 0)] = segs0
        _fetch(0, 1)
        _emit_awindow([0, 1], "01")
        _emit_bwindow([0, 1], 0, 2, "01")
        _emit_awindow([2], "2")
        _emit_bwindow([2], 1, 3, "2")
        _emit_awindow([3], "3")
        _emit_bwindow([3], 2, None, "3", trail_j=3)

    nc.compile()
    return nc


def get_nc():
    if "nc" not in _NC_CACHE:
        _NC_CACHE["nc"] = build_nc()
    return _NC_CACHE["nc"]


def make_in_maps(hidden_states, attention_mask, position_ids, Wq, Wk, Wv, Wo):
    hs = np.asarray(hidden_states, dtype=np.float32)
    pos = np.asarray(position_ids)
    Wq = np.asarray(Wq, dtype=np.float32)
    Wk = np.asarray(Wk, dtype=np.float32)
    Wv = np.asarray(Wv, dtype=np.float32)
    Wo = np.asarray(Wo, dtype=np.float32)
    assert hs.shape == (1, S, HID)
    assert Wq.shape == (HID, HID) and Wk.shape == (HID, 1024)
    assert Wv.shape == (HID, 1024) and Wo.shape == (HID, HID)

    xT = round_fp32r(hs[0].T)

    # 4 distinct diagonal mask tiles: for tile t = 4j + r, the mask in
    # (k, q)-local coords is 0 iff q >= 128 r + k, independent of j.
    kloc = np.arange(128, dtype=np.int32)[:, None]
    qloc = np.arange(512, dtype=np.int32)[None, :]
    m4 = np.stack([
        np.where(qloc >= 128 * r + kloc, np.float32(0),
                 np.float32(-1e9) / np.float32(SCALE))
        .astype(ml_dtypes.bfloat16)
        for r in range(4)
    ])

    p = pos[0].astype(np.float32)
    inv = (1.0 / (10000.0 ** (np.arange(0, D, 2, dtype=np.float32)
                              / np.float32(D)))).astype(np.float32)
    freqs = p[:, None] * inv[None, :]
    emb = np.concatenate([freqs, freqs], axis=1)        # (S, 128)
    cosT = np.ascontiguousarray(np.cos(emb).T).astype(ml_dtypes.bfloat16)
    sinT = np.sin(emb).T.astype(np.float32)
    sinTf = sinT.copy()
    sinTf[:64] *= np.float32(-1.0)
    sinTf = np.ascontiguousarray(sinTf).astype(ml_dtypes.bfloat16)
    ones = np.ones((128, 1), dtype=ml_dtypes.bfloat16)

    in_maps = []
    for c in range(NCORES):
        in_maps.append({
            "xT": xT,
            "wb": np.concatenate(
                [Wq[:, 512 * c:512 * (c + 1)],
                 Wk[:, 128 * c:128 * (c + 1)],
                 Wv[:, 128 * c:128 * (c + 1)]],
                axis=1).astype(ml_dtypes.bfloat16),
            "wob": Wo[512 * c:512 * (c + 1), :].astype(ml_dtypes.bfloat16),
            "mask4": m4,
            "xh": hs[0, 0:512, 0:512].T.astype(ml_dtypes.bfloat16).copy(),
            "cosT": cosT,
            "sinTf": sinTf,
            "ones": ones,
        })
    return in_maps


def kernel(hidden_states, attention_mask, position_ids, Wq, Wk, Wv, Wo):
    # The axon NTFF trace hook isn't shipped in this container; make sure a
    # stray BASS_TRACE in the environment can't route us onto that path.
    os.environ["BASS_NEVER_TRACE"] = "1"
    in_maps = make_in_maps(hidden_states, attention_mask, position_ids,
                           Wq, Wk, Wv, Wo)
    nc = get_nc()
    res = run_bass_kernel_spmd(nc, in_maps, list(range(NCORES)))
    acc = np.zeros((S, HID), dtype=np.float64)
    for c in range(NCORES):
        acc += res.results[c]["y"].astype(np.float64)
    return acc.astype(np.float32)[None]


# revision 65
# speedup vs baseline: 1.0070x; 1.0070x over previous
"""Grouped-query attention (B=1, S=2048, HID=4096, 32 q-heads / 8 kv-heads,
D=128, RoPE, additive causal mask) on 8 Trainium2 NeuronCores.

Sharding: tensor-parallel over heads. Core c owns 4 q-heads (columns
512c:512c+512 of Wq), kv-head c (columns 128c:128c+128 of Wk/Wv), and rows
512c:512c+512 of Wo. Each core emits a full-shape partial of the output
projection in bf16; the host sums the 8 partials (the "all-reduce" of the
row-sharded Wo matmul).

Fully software-pipelined per-core schedule over 512-query slices j = 0..3,
window order A0 A1 | B0B1+C0 | A2 | B2+C1 | A3 | B3+C2+C3:

  A(j)  x^T chunks streamed from DRAM interleaved with the (bf16) combined
        QKV weight; weights cast up to f32r on the Activation engine; the
        first chunk runs straight off the bf16 weights so the PE starts
        ~2.5us in. Q^T/K^T/V^T accumulate in 6 PSUM banks; at the drain, the
        pool close is gated by the last PSUM read, so q0/K/q1 RoPE
        multiplies run PSUM-first on DVE while q2/q3 are staged out through
        Act copies (straight + half-swapped) and RoPE'd on the idle GpSimd
        engine; V transposes on the PE fill the drain latency.
  B(j)  attention (scores transposed so exp lands in P^T layout; P tiles
        bf16; pair-summed softmax denominator: DVE adds + ones-matmuls on
        half the tiles; 1/l broadcast across partitions on GpSimd; diagonal
        tiles t>=4j+2 at half width; depth-4 software pipeline streamed
        continuously across head/slice boundaries). Interleaved as PE
        filler: the previous slice's output projection C(j-1) -- its bf16
        matmuls keep the PE busy while the Activation engine works through
        exp tiles -- plus x prefetches for A(j+1). B0 and B1 share one
        window so B1+C0 work hides B0's Act-bound portion.

PSUM budget per window: A: 6 accumulators (+2 transpose banks at drains);
B: 3 score + 1 lsum + 2 out + 2 yproj banks.

Measured (TimelineSim cost model, matches the HW-exec contract of test.py):
606us baseline -> 395us; hardware rel err vs the f32 reference: 3.3e-3.
"""
import os

import numpy as np
import ml_dtypes
from contextlib import ExitStack

import concourse.tile as tile
from concourse import bacc, mybir
from concourse.bass_utils import run_bass_kernel_spmd
from concourse.masks import make_identity

F32 = mybir.dt.float32
F32R = mybir.dt.float32r
BF16 = mybir.dt.bfloat16
EXP = mybir.ActivationFunctionType.Exp

S = 2048
HID = 4096
D = 128
NCORES = 8
NHQ = 4                      # q heads per core
SCALE = float(D) ** -0.5
ST = S // 128                # 16 s-tiles
SL = S // 512                # 4 s-slices
KT = HID // 128              # 32 hidden k-tiles
NO = HID // 512              # 8 output column slices
NG = KT // 4                 # 8 hidden chunk-groups of 4 k-tiles
CHK = 4                      # k-tiles per fetch chunk
NCH = KT // CHK              # 16 fetch chunks per slice
DEPTH = 4                    # phase-B software pipeline depth

_NC_CACHE = {}


def round_fp32r(a: np.ndarray) -> np.ndarray:
    """Round fp32 to fp32r (1s+8e+11m, top 20 bits), round-to-nearest-even."""
    u = np.ascontiguousarray(a, dtype=np.float32).view(np.uint32)
    low = u & np.uint32(0xFFF)
    base = u & np.uint32(0xFFFFF000)
    lsb = (u >> np.uint32(12)) & np.uint32(1)
    up = (low > np.uint32(0x800)) | ((low == np.uint32(0x800)) & (lsb == np.uint32(1)))
    return (base + np.where(up, np.uint32(0x1000), np.uint32(0))).view(np.float32)


def build_nc():
    nc = bacc.Bacc("TRN2", target_bir_lowering=False, debug=False,
                   num_devices=NCORES)
    xT = nc.dram_tensor("xT", [HID, S], F32R, kind="ExternalInput").ap()
    wb = nc.dram_tensor("wb", [HID, 128 * NHQ + 2 * D], BF16,
                        kind="ExternalInput").ap()
    wob = nc.dram_tensor("wob", [128 * NHQ, HID], BF16, kind="ExternalInput").ap()
    mask4 = nc.dram_tensor("mask4", [4, 128, 512], BF16, kind="ExternalInput").ap()
    cosT = nc.dram_tensor("cosT", [128, S], BF16, kind="ExternalInput").ap()
    sinTf = nc.dram_tensor("sinTf", [128, S], BF16, kind="ExternalInput").ap()
    ones = nc.dram_tensor("ones", [128, 1], BF16, kind="ExternalInput").ap()
    xh = nc.dram_tensor("xh", [512, 512], BF16, kind="ExternalInput").ap()
    y = nc.dram_tensor("y", [S, HID], BF16, kind="ExternalOutput").ap()

    with tile.TileContext(nc) as tc, ExitStack() as ctx:
        const = ctx.enter_context(tc.tile_pool(name="const", bufs=1))
        ones_sb = const.tile([128, 1], BF16)
        ident = const.tile([128, 128], F32)
        mask_sb = const.tile([128, 4, 512], BF16)
        kt_sl = [const.tile([128, 512], F32R, tag=f"kt{j}", name=f"kt{j}")
                 for j in range(SL)]
        v_sl = [const.tile([128, 512], BF16, tag=f"v{j}", name=f"v{j}")
                for j in range(SL)]
        wb_sb = const.tile([128, KT, 128 * NHQ + 2 * D], BF16)

        xtp = ctx.enter_context(tc.tile_pool(name="xtp", bufs=3))
        wkvc = ctx.enter_context(tc.tile_pool(name="wkvc", bufs=3))
        ropes = ctx.enter_context(tc.tile_pool(name="ropes", bufs=5))
        vtmp = ctx.enter_context(tc.tile_pool(name="vtmp", bufs=1))
        qtp = ctx.enter_context(tc.tile_pool(name="qtp", bufs=2))
        outTp = ctx.enter_context(tc.tile_pool(name="outTp", bufs=2))
        ptbp = ctx.enter_context(tc.tile_pool(name="ptbp", bufs=9))
        prp = ctx.enter_context(tc.tile_pool(name="prp", bufs=3))
        rbcp = ctx.enter_context(tc.tile_pool(name="rbcp", bufs=1))
        wop = ctx.enter_context(tc.tile_pool(name="wop", bufs=3))
        csp = ctx.enter_context(tc.tile_pool(name="csp", bufs=2))
        ysp = ctx.enter_context(tc.tile_pool(name="ysp", bufs=3))

        make_identity(nc, ident[:])

        wbr = wb.rearrange("(t p) f -> p t f", p=128)
        wor = wob.rearrange("(k p) o -> p k o", p=128)
        casts = {}
        cs_sl = {}
        sn_sl = {}
        qt_sl = {}
        outT_sl = {}
        wo_t = {}

        def _fetch(j, g):
            """DMA (j0: weights +) x chunk; cast weight chunk up to f32r."""
            sk = slice(CHK * g, CHK * (g + 1))
            if j == 0:
                nc.sync.dma_start(out=wb_sb[:, sk, :], in_=wbr[:, sk, :])
            xt = xtp.tile([128, CHK, 512], F32R, tag="xt", name="xt")
            nc.sync.dma_start(
                out=xt[:],
                in_=xT[128 * CHK * g:128 * CHK * (g + 1),
                       512 * j:512 * (j + 1)]
                .rearrange("(t p) m -> p t m", p=128),
            )
            if j == 0 and 3 <= g <= 4:
                # constants are first needed at the j0 RoPE drain / B(0);
                # keep them off the prologue's critical DMA path
                if g == 3:
                    nc.sync.dma_start(out=ones_sb[:], in_=ones[:])
                else:
                    nc.sync.dma_start(out=mask_sb[:],
                                      in_=mask4.rearrange("t p q -> p t q"))
            if g == 5:
                cs_sl[j] = csp.tile([128, 512], BF16, tag="cs", name="cs")
                nc.sync.dma_start(out=cs_sl[j][:],
                                  in_=cosT[:, 512 * j:512 * (j + 1)])
            elif g == 6:
                sn_sl[j] = csp.tile([128, 512], BF16, tag="sn", name="sn")
                nc.sync.dma_start(out=sn_sl[j][:],
                                  in_=sinTf[:, 512 * j:512 * (j + 1)])
            # weight cast on Act: DVE is busy with RoPE drains and must
            # not delay the next slice's first matmuls
            wc = wkvc.tile([128, CHK, 128 * NHQ + 2 * D], F32R, tag="wc",
                           name="wc")
            nc.scalar.copy(wc[:], wb_sb[:, sk, :])
            casts[(j, g)] = [(xt, wc[:, :, 0:512], wc[:, :, 512:640],
                              wc[:, :, 640:768], 0, CHK)]

        def _prefetch_wo(jm1, n):
            wt = wop.tile([128, NHQ, 512], BF16, tag="wo", name="wo")
            nc.sync.dma_start(out=wt[:], in_=wor[:, :, 512 * n:512 * (n + 1)])
            wo_t[(jm1, n)] = wt

        def _emit_cblock(jm1, n, psC, split_dma=False, act_copies=False):
            """Output projection for q-slice jm1, output columns 512n:512n+512."""
            wt = wo_t.pop((jm1, n))
            ys = ysp.tile([128, 4, 512], BF16, tag="ys", name="ys")
            for sq2 in range(4):
                yp = psC.tile([128, 512], F32, tag="yp", name="yp")
                for h in range(NHQ):
                    nc.tensor.matmul(
                        yp[:], outT_sl[(h, jm1)][:, 128 * sq2:128 * (sq2 + 1)],
                        wt[:, h, :], start=(h == 0), stop=(h == NHQ - 1))
                if sq2 % 2 == 0 and not act_copies:
                    nc.vector.tensor_copy(ys[:, sq2, :], yp[:])
                else:
                    nc.scalar.copy(ys[:, sq2, :], yp[:])
                if split_dma:
                    sq = 4 * jm1 + sq2
                    nc.sync.dma_start(
                        out=y[128 * sq:128 * (sq + 1),
                              512 * n:512 * (n + 1)],
                        in_=ys[:, sq2, :])
            if not split_dma:
                nc.sync.dma_start(
                    out=y[512 * jm1:512 * (jm1 + 1), 512 * n:512 * (n + 1)]
                    .rearrange("(t p) o -> p t o", p=128),
                    in_=ys[:])

        # ---------------- phase B emission helpers ---------------------------
        bstate = {}

        def _off(j, t):
            # diagonal tiles t >= 4j+2 only need the upper half q range
            return 256 if t >= 4 * j + 2 else 0

        def _bseq(j, h):
            diag = [4 * j, 4 * j + 1, 4 * j + 2, 4 * j + 3]
            nond = list(range(4 * j))
            # h=0 starts with non-diagonal tiles: they only touch earlier
            # slices' K/V, so B(j) can begin while slice j still drains
            return nond + diag if h == 0 else diag + nond

        def _blead(j, h, pos, psB, psL, psO):
            seq = _bseq(j, h)
            t = seq[pos]
            if pos == 0:
                bstate[(j, h)] = {
                    "lps": psL.tile([1, 512], F32, tag="lps", name="lps"),
                    "ops": psO.tile([128, 512], F32, tag="ops", name="ops"),
                    "pts": {}, "pairs": {},
                }
            st = bstate[(j, h)]
            o = _off(j, t)
            scp = psB.tile([128, 512], F32, tag="scp", name="scp")
            nc.tensor.matmul(
                scp[:, o:512], kt_sl[t // 4][:, 128 * (t % 4):128 * (t % 4 + 1)],
                qt_sl[(h, j)][:, o:512],
                start=True, stop=True, skip_group_check=True)
            if t >= 4 * j:  # diagonal block: additive mask
                nc.vector.tensor_add(
                    scp[:, o:512], scp[:, o:512], mask_sb[:, t - 4 * j, o:512])
            ptb = ptbp.tile([128, 512], BF16, tag="ptb", name="ptb")
            nc.scalar.activation(ptb[:, o:512], scp[:, o:512],
                                 EXP, bias=0.0, scale=SCALE)
            st["pts"][t] = ptb
            if pos % 2 == 1:
                t0 = seq[pos - 1]
                o2 = _off(j, t0)  # pair partners share width by construction
                pp = prp.tile([128, 512], BF16, tag="pp", name="pp")
                nc.vector.tensor_add(pp[:, o2:512], st["pts"][t0][:, o2:512],
                                     st["pts"][t][:, o2:512])
                st["pairs"][pos // 2] = pp

        def _btrail(j, h, pos):
            seq = _bseq(j, h)
            t = seq[pos]
            st = bstate[(j, h)]
            o = _off(j, t)
            nc.tensor.matmul(
                st["ops"][:, o:512], v_sl[t // 4][:, 128 * (t % 4):128 * (t % 4 + 1)],
                st["pts"].pop(t)[:, o:512],
                start=(pos == 0), stop=(pos == len(seq) - 1),
                skip_group_check=True)
            if pos % 2 == 1:
                o2 = _off(j, seq[pos - 1])
                nc.tensor.matmul(
                    st["lps"][:, o2:512], ones_sb[:],
                    st["pairs"].pop(pos // 2)[:, o2:512],
                    start=(pos == 1), stop=(pos == len(seq) - 1),
                    skip_group_check=True)
            if pos == len(seq) - 1:
                rsb = rbcp.tile([1, 512], F32, tag="rsb", name="rsb")
                nc.vector.reciprocal(rsb[:], st["lps"][:])
                rb = rbcp.tile([128, 512], F32, tag="rb", name="rb")
                nc.gpsimd.partition_broadcast(rb[:], rsb[:])
                ot = outTp.tile([128, 512], BF16, tag=f"outT{h}",
                                name=f"outT{h}")
                nc.vector.tensor_mul(ot[:], st["ops"][:], rb[:])
                outT_sl[(h, j)] = ot
                del bstate[(j, h)]

        def _emit_awindow(jlist, wname):
            """Projection chunks + drains for the slices in jlist."""
            stA = ExitStack()
            psA = stA.enter_context(
                tc.tile_pool(name=f"psA{wname}", bufs=1, space="PSUM"))
            chunks = [(j, g) for j in jlist for g in range(NCH)]
            for ci, (j, g) in enumerate(chunks):
                if g == 0:
                    qps = [psA.tile([128, 512], F32, tag=f"qps{f}",
                                    name=f"qps{f}") for f in range(NHQ)]
                    kps = psA.tile([128, 512], F32, tag="kps", name="kps")
                    vps = psA.tile([128, 512], F32, tag="vps", name="vps")
                if ci + 2 < len(chunks) and chunks[ci + 2] not in casts:
                    _fetch(*chunks[ci + 2])
                for xt, wqc, wkc, wvc, k0, nk in casts.pop((j, g)):
                    for kk in range(nk):
                        k = CHK * g + k0 + kk
                        rhs = xt[:, kk, :]
                        st, sp = (k == 0), (k == KT - 1)
                        if not sp:
                            for f in range(NHQ):
                                nc.tensor.matmul(
                                    qps[f][:],
                                    wqc[:, kk, 128 * f:128 * (f + 1)],
                                    rhs, start=st, stop=sp)
                            nc.tensor.matmul(kps[:], wkc[:, kk, :], rhs,
                                             start=st, stop=sp)
                            nc.tensor.matmul(vps[:], wvc[:, kk, :], rhs,
                                             start=st, stop=sp)
                        else:
                            # last k-tile: K/V first so their drains (kt
                            # RoPE on DVE, vts copy on Act) start earlier
                            nc.tensor.matmul(kps[:], wkc[:, kk, :], rhs,
                                             start=st, stop=sp)
                            nc.tensor.matmul(vps[:], wvc[:, kk, :], rhs,
                                             start=st, stop=sp)
                            for f in range(NHQ):
                                nc.tensor.matmul(
                                    qps[f][:],
                                    wqc[:, kk, 128 * f:128 * (f + 1)],
                                    rhs, start=st, stop=sp)
                if g == NCH - 1:
                    # Drains. The pool close (and thus the next window) waits
                    # on the last PSUM read, so emit all PSUM-reading muls
                    # first (q3's on the idle GpSimd engine, in parallel) and
                    # defer the rotate-half adds; V transposes on the PE fill
                    # the drain latency.
                    cs = cs_sl[j][:]
                    sn = sn_sl[j][:]
                    vts = vtmp.tile([128, 512], F32, tag="vts", name="vts")
                    nc.scalar.copy(vts[:], vps[:])
                    for h in range(NHQ):
                        qt_sl[(h, j)] = qtp.tile([128, 512], F32R,
                                                 tag=f"qt{h}", name=f"qt{h}")

                    def _muls(eng, dst, src):
                        rot = ropes.tile([128, 512], BF16, tag="rot",
                                         name="rot")
                        eng.tensor_mul(dst[:], src[:], cs)
                        eng.tensor_mul(rot[0:64, :], src[64:128, :],
                                       sn[0:64, :])
                        eng.tensor_mul(rot[64:128, :], src[0:64, :],
                                       sn[64:128, :])
                        return rot

                    # GpSimd cannot read PSUM, and SBUF-SBUF ops need equal
                    # base partitions: stage q2/q3 twice via Act copies from
                    # PSUM (straight + half-swapped), then RoPE them on the
                    # idle Pool engine with fully-aligned ops. This takes
                    # their PSUM reads off the DVE critical path that gates
                    # the pool close (and thus the next window).
                    def _stage_pool_rope(f):
                        qu = vtmp.tile([128, 512], BF16, tag=f"q{f}u",
                                       name=f"q{f}u")
                        qs = vtmp.tile([128, 512], BF16, tag=f"q{f}s",
                                       name=f"q{f}s")
                        nc.scalar.copy(qu[:], qps[f][:])
                        nc.scalar.copy(qs[0:64, :], qps[f][64:128, :])
                        nc.scalar.copy(qs[64:128, :], qps[f][0:64, :])
                        qtf = qt_sl[(f, j)]
                        rotf = ropes.tile([128, 512], BF16, tag="rot",
                                          name="rot")
                        nc.gpsimd.tensor_mul(qtf[:], qu[:], cs)
                        nc.gpsimd.tensor_mul(rotf[:], qs[:], sn)
                        nc.gpsimd.tensor_add(qtf[:], qtf[:], rotf[:])

                    rot0 = _muls(nc.vector, qt_sl[(0, j)], qps[0])
                    rotk = _muls(nc.vector, kt_sl[j], kps)
                    with tc.tile_pool(name=f"psAv{j}", bufs=2,
                                      space="PSUM") as psAv:
                        for t2 in range(4):
                            vtp = psAv.tile([128, 128], F32, tag="vtp",
                                            name="vtp")
                            nc.tensor.transpose(
                                vtp[:], vts[:, 128 * t2:128 * (t2 + 1)],
                                ident[:])
                            nc.scalar.copy(
                                v_sl[j][:, 128 * t2:128 * (t2 + 1)], vtp[:])
                    rot1 = _muls(nc.vector, qt_sl[(1, j)], qps[1])
                    _stage_pool_rope(3)
                    _stage_pool_rope(2)
                    nc.vector.tensor_add(qt_sl[(0, j)][:], qt_sl[(0, j)][:],
                                         rot0[:])
                    nc.vector.tensor_add(kt_sl[j][:], kt_sl[j][:], rotk[:])
                    nc.vector.tensor_add(qt_sl[(1, j)][:], qt_sl[(1, j)][:],
                                         rot1[:])
            stA.close()

        def _emit_bwindow(jlist, cfill_j, fetch_j, wname, trail_j=None):
            """Attention for slices in jlist, with output-projection blocks
            of slice cfill_j and x prefetches for slice fetch_j as PE
            filler."""
            stB = ExitStack()
            psB = stB.enter_context(
                tc.tile_pool(name=f"psB{wname}", bufs=3, space="PSUM"))
            psL = stB.enter_context(
                tc.tile_pool(name=f"psL{wname}", bufs=1, space="PSUM"))
            psO = stB.enter_context(
                tc.tile_pool(name=f"psO{wname}", bufs=2, space="PSUM"))
            psC = stB.enter_context(
                tc.tile_pool(name=f"psC{wname}", bufs=2, space="PSUM"))

            items = [(j, h, pos) for j in jlist for h in range(NHQ)
                     for pos in range(4 * j + 4)]
            fills = []
            if cfill_j is not None:
                for n in range(NO):
                    fills.append(("cblock", cfill_j, n))
                _prefetch_wo(cfill_j, 0)
            if fetch_j is not None:
                fills.append(("fetch", fetch_j, 0))
                fills.append(("fetch", fetch_j, 1))
            # C blocks of slice jlist[0]-1 may only start once that slice's
            # outT is complete; when filling with jlist[0]'s own C work
            # (merged B0+B1 window), wait for the first slice to trail out
            fill_start = 0
            if cfill_j is not None and cfill_j in jlist:
                fill_start = 4 * (4 * cfill_j + 4) + DEPTH
            every = max(1, (len(items) - fill_start) // max(1, len(fills) + 1))

            held = [f for f in fills if f[0] == "cblock"][-2:]
            for f in held:
                fills.remove(f)
            fi = 0
            for idx in range(len(items) + DEPTH):
                if idx < len(items):
                    _blead(*items[idx], psB, psL, psO)
                if (idx >= fill_start and fi < len(fills)
                        and (idx - fill_start) % every == every - 1):
                    kind, a1, a2 = fills[fi]
                    fi += 1
                    if kind == "cblock":
                        if a2 + 1 < NO:
                            _prefetch_wo(a1, a2 + 1)
                        _emit_cblock(a1, a2, psC)
                    else:
                        _fetch(a1, a2)
                if idx >= DEPTH:
                    _btrail(*items[idx - DEPTH])
            fills.extend(held)
            while fi < len(fills):
                kind, a1, a2 = fills[fi]
                fi += 1
                if kind == "cblock":
                    if a2 + 1 < NO:
                        _prefetch_wo(a1, a2 + 1)
                    # trailing blocks: keep their PSUM drains off the busy
                    # DVE so the next A window isn't held up
                    _emit_cblock(a1, a2, psC, act_copies=True)
                else:
                    _fetch(a1, a2)
            if trail_j is not None:
                # final slice's own output projection, inside this window
                _prefetch_wo(trail_j, 0)
                for n in range(NO):
                    if n + 1 < NO:
                        _prefetch_wo(trail_j, n + 1)
                    _emit_cblock(trail_j, n, psC, split_dma=(n >= NO - 2))
            stB.close()

        # ===================== main schedule =================================
        # A0 A1 | B0+B1 (+C0 fills) | A2 | B2 (+C1) | A3 | B3 (+C2) | C3
        # chunk (0,0) runs fully in bf16 straight off the resident bf16
        # weights and a small bf16 x slice -- no casts on the critical path,
        # so the PE starts ~2.5us in instead of ~7
        halfp = ctx.enter_context(tc.tile_pool(name="halfp", bufs=1))
        segs0 = []
        for hi in range(2):
            sk = slice(2 * hi, 2 * hi + 2)
            nc.sync.dma_start(out=wb_sb[:, sk, :], in_=wbr[:, sk, :])
            xth = halfp.tile([128, 2, 512], BF16, tag=f"xth{hi}",
                             name=f"xth{hi}")
            nc.sync.dma_start(
                out=xth[:],
                in_=xh[256 * hi:256 * (hi + 1), :]
                .rearrange("(t p) m -> p t m", p=128))
            segs0.append((xth, wb_sb[:, sk, 0:512], wb_sb[:, sk, 512:640],
                          wb_sb[:, sk, 640:768], 2 * hi, 2))
        casts[(0, 0)] = segs0
        _fetch(0, 1)
        _emit_awindow([0, 1], "01")
        _emit_bwindow([0, 1], 0, 2, "01")
        _emit_awindow([2], "2")
        _emit_bwindow([2], 1, 3, "2")
        _emit_awindow([3], "3")
        _emit_bwindow([3], 2, None, "3", trail_j=3)

    nc.compile()
    return nc


def get_nc():
    if "nc" not in _NC_CACHE:
        _NC_CACHE["nc"] = build_nc()
    return _NC_CACHE["nc"]


def make_in_maps(hidden_states, attention_mask, position_ids, Wq, Wk, Wv, Wo):
    hs = np.asarray(hidden_states, dtype=np.float32)
    pos = np.asarray(position_ids)
    Wq = np.asarray(Wq, dtype=np.float32)
    Wk = np.asarray(Wk, dtype=np.float32)
    Wv = np.asarray(Wv, dtype=np.float32)
    Wo = np.asarray(Wo, dtype=np.float32)
    assert hs.shape == (1, S, HID)
    assert Wq.shape == (HID, HID) and Wk.shape == (HID, 1024)
    assert Wv.shape == (HID, 1024) and Wo.shape == (HID, HID)

    xT = round_fp32r(hs[0].T)

    # 4 distinct diagonal mask tiles: for tile t = 4j + r, the mask in
    # (k, q)-local coords is 0 iff q >= 128 r + k, independent of j.
    kloc = np.arange(128, dtype=np.int32)[:, None]
    qloc = np.arange(512, dtype=np.int32)[None, :]
    m4 = np.stack([
        np.where(qloc >= 128 * r + kloc, np.float32(0),
                 np.float32(-1e9) / np.float32(SCALE))
        .astype(ml_dtypes.bfloat16)
        for r in range(4)
    ])

    p = pos[0].astype(np.float32)
    inv = (1.0 / (10000.0 ** (np.arange(0, D, 2, dtype=np.float32)
                              / np.float32(D)))).astype(np.float32)
    freqs = p[:, None] * inv[None, :]
    emb = np.concatenate([freqs, freqs], axis=1)        # (S, 128)
    cosT = np.ascontiguousarray(np.cos(emb).T).astype(ml_dtypes.bfloat16)
    sinT = np.sin(emb).T.astype(np.float32)
    sinTf = sinT.copy()
    sinTf[:64] *= np.float32(-1.0)
    sinTf = np.ascontiguousarray(sinTf).astype(ml_dtypes.bfloat16)
    ones = np.ones((128, 1), dtype=ml_dtypes.bfloat16)

    in_maps = []
    for c in range(NCORES):
        in_maps.append({
            "xT": xT,
            "wb": np.concatenate(
                [Wq[:, 512 * c:512 * (c + 1)],
                 Wk[:, 128 * c:128 * (c + 1)],
                 Wv[:, 128 * c:128 * (c + 1)]],
                axis=1).astype(ml_dtypes.bfloat16),
            "wob": Wo[512 * c:512 * (c + 1), :].astype(ml_dtypes.bfloat16),
            "mask4": m4,
            "xh": hs[0, 0:512, 0:512].T.astype(ml_dtypes.bfloat16).copy(),
            "cosT": cosT,
            "sinTf": sinTf,
            "ones": ones,
        })
    return in_maps


def kernel(hidden_states, attention_mask, position_ids, Wq, Wk, Wv, Wo):
    # The axon NTFF trace hook isn't shipped in this container; make sure a
    # stray BASS_TRACE in the environment can't route us onto that path.
    os.environ["BASS_NEVER_TRACE"] = "1"
    in_maps = make_in_maps(hidden_states, attention_mask, position_ids,
                           Wq, Wk, Wv, Wo)
    nc = get_nc()
    res = run_bass_kernel_spmd(nc, in_maps, list(range(NCORES)))
    acc = np.zeros((S, HID), dtype=np.float64)
    for c in range(NCORES):
        acc += res.results[c]["y"].astype(np.float64)
    return acc.astype(np.float32)[None]
